# revision 38
# baseline (speedup 1.0000x reference)
"""GQA attention (B=2, L=2048, E=2048, 32 q-heads / 8 kv-heads, D=64) on 8 trn2
NeuronCores.

Sharding: tensor-parallel over kv-heads. Core h owns kv-head h: the 4 q-heads
4h..4h+3 (W_Q rows 256h:256h+256), W_K/W_V rows 64h:64h+64, and W_O columns
256h:256h+256. Each core computes a full-shape partial output
(x @ Wq_h -> attention -> @ Wo_h^T); the host sums the 8 partials (the
"all-reduce") and transposes back.

Device kernel layout notes:
  - fp16 operands everywhere (PSUM accumulation stays fp32): halves DMA/SBUF
    vs fp32 and keeps matmuls at the full 1 column/cycle stream rate.
  - x is fed pre-transposed (B, E, L) so the QKV projections consume it with
    the contraction dim (E) on partitions. Q/K are produced transposed
    (dims on partitions, tokens free), so scores are computed transposed:
    S^T[k, q] per 128-ktoken tile. Softmax needs no max pass (scores ~ N(0,1))
    and no transposes; the denominator comes from a ones-column appended to V.
  - Score matmuls have K=64 contraction, so two heads are packed into the PE
    array with row tiling: the even head of a pair streams through rows 0:64
    (stationary = K^T at partitions 0:64), the odd head through rows 64:128
    (stationary = a DMA-duplicated K^T at partitions 64:128, moving = odd Q
    which the QKV projection already leaves at partitions 64:128). The two
    matmuls execute concurrently -> ~2x on the score phase.
  - The attention inner loop is software-pipelined: scores for k-tile kt+1
    issue before the attn@V matmuls of k-tile kt, with filler PE work from
    other phases in between, so the PE never stalls on the ACT-engine exp.
  - Normalization: denominator rows ([1,512] each) are gathered by tiny
    SBUF->SBUF DMAs into a [4,512] tile per (b, q-chunk), one batched DVE
    reciprocal, then broadcast across 64 partitions with small one-hot
    matmuls on PE and applied with one DVE multiply per head.
  - Emission is software-pipelined across phases via a filler deque: QKV for
    batch b+1 and the per-q-chunk output projection slices (as soon as their
    outT q-chunk is normalized) are interleaved into the ACT(exp)-bound
    attention loop so the PE never idles long enough for the HAM clock gate
    to re-throttle it to 1.2 GHz.
"""

from collections import deque

import numpy as np

B, L, E = 2, 2048, 2048
HKV, D, G = 8, 64, 4          # kv heads (=cores), head dim, q-heads per core
QD = G * D                    # 256 q dims per core
N_CORES = 8
EC = E // 128                 # 16 contraction chunks for projections
NQC = L // 512                # 4 q-token chunks of 512
KT = L // 128                 # 16 k-token tiles of 128

_cache = {}


def _build_nc():
    import concourse.bass as bass
    import concourse.mybir as mybir
    import concourse.tile as tile
    from concourse import bacc
    from contextlib import ExitStack

    f32 = mybir.dt.float32
    f16 = mybir.dt.float16
    VW = D + 1  # V columns + the ones column (softmax denominator trick)

    nc = bacc.Bacc("TRN2", target_bir_lowering=False, debug=False)
    xT_d = nc.declare_dram_parameter("xT", [B, E, L], f16, isOutput=False)
    wq_d = nc.declare_dram_parameter("wq", [E, QD], f16, isOutput=False)
    wkv_d = nc.declare_dram_parameter("wkv", [E, 2 * D], f16, isOutput=False)
    wo_d = nc.declare_dram_parameter("wo", [QD, E], f16, isOutput=False)
    ident_d = nc.declare_dram_parameter("ident", [128, 128], f16, isOutput=False)
    ones_d = nc.declare_dram_parameter("ones", [1, 128], f16, isOutput=False)
    sel_d = nc.declare_dram_parameter("sel", [4, 4 * 64], f16, isOutput=False)
    out_d = nc.declare_dram_parameter("out", [B, E, L], f16, isOutput=True)

    with ExitStack() as ctx:
        tc = ctx.enter_context(tile.TileContext(nc))
        singles = ctx.enter_context(tc.tile_pool(name="singles", bufs=1))
        xt_pool = ctx.enter_context(tc.tile_pool(name="xtp", bufs=34))
        qt_pool = ctx.enter_context(tc.tile_pool(name="qtp", bufs=2))
        kv_pool = ctx.enter_context(tc.tile_pool(name="kvp", bufs=2))
        kd_pool = ctx.enter_context(tc.tile_pool(name="kdp", bufs=2))
        vs_pool = ctx.enter_context(tc.tile_pool(name="vsp", bufs=2))
        es_pool = ctx.enter_context(tc.tile_pool(name="esp", bufs=4))
        ov_pool = ctx.enter_context(tc.tile_pool(name="ovp", bufs=10))
        rc_pool = ctx.enter_context(tc.tile_pool(name="rcp", bufs=2))
        ot_pool = ctx.enter_context(tc.tile_pool(name="otp", bufs=2))
        nt_pool = ctx.enter_context(tc.tile_pool(name="ntp", bufs=3))
        st_pool = ctx.enter_context(tc.tile_pool(name="stp", bufs=4))
        ps_sc = ctx.enter_context(tc.tile_pool(name="pssc", bufs=2, space="PSUM"))
        ps_va = ctx.enter_context(tc.tile_pool(name="psva", bufs=2, space="PSUM"))
        ps_mm = ctx.enter_context(tc.tile_pool(name="psmm", bufs=2, space="PSUM"))

        # ---- static weights / constants ----
        # DMA order matters: ident (warm-up) and wkv (first projections)
        # must land before the bulk weights; wq/wo are issued later, after
        # the first half of x, from inside gen_qkv_part1(0).
        ident = singles.tile([128, 128], f16)
        nc.sync.dma_start(out=ident, in_=ident_d[:, :])
        sel_sb = singles.tile([4, 4 * 64], f16)
        nc.sync.dma_start(out=sel_sb, in_=sel_d[:, :])
        wkv_sb = singles.tile([128, EC * 2 * D], f16)
        nc.sync.dma_start(
            out=wkv_sb.rearrange("p (e m) -> p e m", e=EC),
            in_=wkv_d.rearrange("(e p) m -> p e m", p=128),
        )
        wq_sb = singles.tile([128, EC * QD], f16)  # e-chunk e at cols [e*256,(e+1)*256)
        wo_sb = [singles.tile([128, E], f16, name=f"wo_sb{kc}")
                 for kc in range(2)]

        # warm-up burst: keep the PE busy during the initial x DMA so the
        # HAM clock gate ramps to 2.4 GHz before the real work arrives
        for _ in range(48):
            wps = ps_mm.tile([128, 128], f16, name="warm", tag="mm")
            nc.tensor.transpose(wps, ident, ident)

        state = {}

        def qkv_group(b, xts, m, hf, t):
            """One [128,512] projection group: accumulate over 16 e-chunks."""
            n = hf * 2 + t
            nsl = slice(n * 512, (n + 1) * 512)
            ps = ps_mm.tile([128, 512], f32, name="ps_qkv", tag="mm")
            for e in range(EC):
                if m < 2:
                    lhsT = wq_sb[:, e * QD + m * 128: e * QD + (m + 1) * 128]
                else:
                    lhsT = wkv_sb[:, e * 2 * D:(e + 1) * 2 * D]
                nc.tensor.matmul(
                    ps, lhsT, xts[hf][e][:, t * 512:(t + 1) * 512],
                    start=(e == 0), stop=(e == EC - 1),
                )
                yield
            st = state[b]
            dst = st["qpair"][m] if m < 2 else st["kvT"]
            nc.vector.tensor_copy(dst[:, nsl], ps)
            if m == 2:
                # duplicate K^T to partitions 64:128 for the row-tiled
                # odd-head score matmuls
                nc.sync.dma_start(
                    out=st["kdup"][64:128, nsl], in_=st["kvT"][0:64, nsl])

        def gen_qkv_part1(b):
            """x DMAs, K/V projection (+K dup), V transpose, first q group."""
            qpair = [qt_pool.tile([128, L], f16, name=f"qpair{p}", tag=f"qpair{p}")
                     for p in range(2)]
            kvT = kv_pool.tile([128, L], f16, name="kvT", tag="kvT")
            kdup = kd_pool.tile([128, L], f16, name="kdup", tag="kdup")
            v_sb = vs_pool.tile([128, KT * VW], f16, name="v_sb", tag="v_sb")
            xts = {}
            state[b] = dict(qpair=qpair, kvT=kvT, kdup=kdup, v_sb=v_sb, xts=xts)
            # ones column (denominator trick) via partition-broadcast DMA
            ones_bcast = bass.AP(
                tensor=ones_d[0:1, 0:KT].tensor, offset=0,
                ap=[[0, 128], [1, KT]])
            nc.sync.dma_start(
                out=v_sb.rearrange("p (k c) -> p k c", c=VW)[:, :, D],
                in_=ones_bcast)
            for hf in range(2):
                xts[hf] = []
                for e in range(EC):
                    xt = xt_pool.tile([128, 1024], f16, name=f"xt_{e}", tag="xt")
                    nc.sync.dma_start(
                        out=xt,
                        in_=xT_d[b, e * 128:(e + 1) * 128,
                                 hf * 1024:(hf + 1) * 1024],
                    )
                    xts[hf].append(xt)
                if b == 0 and hf == 0:
                    # bulk weights ride behind the first x half
                    nc.sync.dma_start(
                        out=wq_sb.rearrange("p (e m) -> p e m", e=EC),
                        in_=wq_d.rearrange("(e p) m -> p e m", p=128),
                    )
                    for kc in range(2):
                        nc.sync.dma_start(
                            out=wo_sb[kc],
                            in_=wo_d[kc * 128:(kc + 1) * 128, :])
            for hf in range(2):
                for t in range(2):
                    yield from qkv_group(b, xts, 2, hf, t)
            for kt in range(KT):
                psv = ps_mm.tile([128, 64], f16, name="ps_vt", tag="mm")
                nc.tensor.transpose(
                    psv, kvT[64:128, kt * 128:(kt + 1) * 128],
                    ident[64:128, 64:128])
                nc.vector.tensor_copy(
                    v_sb[:, kt * VW: kt * VW + D], psv)
                yield
            yield from qkv_group(b, xts, 0, 0, 0)  # (pair0, qc0)

        def gen_q_group(b, pair, qc):
            """One q projection group; attention (qc,pair) guards on it."""
            yield from qkv_group(b, state[b]["xts"], pair, qc // 2, qc % 2)

        filler = deque()

        def pull(k):
            n = 0
            while n < k and filler:
                try:
                    next(filler[0])
                    n += 1
                except StopIteration:
                    filler.popleft()

        def drain_through(gen):
            """Emit deque items in order until `gen` is exhausted."""
            if gen not in filler:
                return
            while filler:
                head = filler[0]
                try:
                    next(head)
                except StopIteration:
                    filler.popleft()
                    if head is gen:
                        return

        def gen_oproj_qc(b, qc):
            """Output projection for one 512-token chunk of batch b."""
            outT = state.pop((b, "outT", qc))
            nsl = slice(qc * 512, (qc + 1) * 512)
            for m in range(EC):
                msl = slice(m * 128, (m + 1) * 128)
                ps = ps_mm.tile([128, 512], f32, name="ps_op", tag="mm")
                for kc in range(2):
                    nc.tensor.matmul(
                        ps, wo_sb[kc][:, msl], outT[kc][:, :],
                        start=(kc == 0), stop=(kc == 1),
                    )
                    yield
                stg = st_pool.tile([128, 512], f16, name="stg", tag="stg")
                nc.vector.tensor_copy(stg, ps)
                nc.gpsimd.dma_start(out=out_d[b, msl, nsl], in_=stg)

        def gen_attn(b, need):
            """Attention for batch b; need[(pair,qc)] = q-group generators
            that must be fully emitted before that section's scores."""
            st = state[b]
            qpair, kvT, kdup, v_sb = (
                st["qpair"], st["kvT"], st["kdup"], st["v_sb"])
            for qc in range(NQC):
                qsl = slice(qc * 512, (qc + 1) * 512)
                outT = [ot_pool.tile([128, 512], f16, name=f"oT{p}",
                                     tag=f"outT{p}q{qc}")
                        for p in range(2)]
                state[(b, "outT", qc)] = outT
                recin = rc_pool.tile([4, 512], f16, name="recin", tag="recin")
                ovs = []
                for pair in range(2):
                    g = need.pop((pair, qc), None)
                    if g is not None:
                        drain_through(g)
                    vacc = [
                        ps_va.tile([VW, 512], f32, name=f"vacc{hh}",
                                   tag="vacc")
                        for hh in range(2)]

                    def spair(jj, r):
                        # per head: scores for k-tile 2jj+r; the even head
                        # streams PE rows 0:64, the odd head rows 64:128 --
                        # adjacent emission makes the pair run concurrently
                        kt = 2 * jj + r
                        ksl = slice(kt * 128, (kt + 1) * 128)
                        csl = slice(r * 512, (r + 1) * 512)
                        nc.tensor.matmul(
                            ssc[0][:, csl], kvT[0:64, ksl],
                            qpair[pair][0:64, qsl], start=True, stop=True)
                        nc.tensor.matmul(
                            ssc[1][:, csl], kdup[64:128, ksl],
                            qpair[pair][64:128, qsl], start=True, stop=True)

                    def avpair(jj, r):
                        es2 = es_q[0]
                        kt = 2 * jj + r
                        for hh in range(2):
                            nc.tensor.matmul(
                                vacc[hh][0:VW, :],
                                v_sb[:, kt * VW:(kt + 1) * VW],
                                es2[hh][:, r * 512:(r + 1) * 512],
                                start=(kt == 0), stop=(kt == KT - 1),
                            )

                    es_q = deque()
                    for jj in range(KT // 2 + 1):
                        if jj < KT // 2:
                            ssc = [ps_sc.tile([128, 1024], f32,
                                              name=f"ssc{hh}", tag="ssc")
                                   for hh in range(2)]
                            spair(jj, 0)
                            spair(jj, 1)
                            es2 = []
                            for hh in range(2):
                                es = es_pool.tile([128, 1024], f16, name="es",
                                                  tag="es")
                                nc.scalar.activation(
                                    es, ssc[hh],
                                    mybir.ActivationFunctionType.Exp,
                                    scale=0.125)
                                es2.append(es)
                            es_q.append(es2)
                        pull(2)
                        if jj > 0:
                            avpair(jj - 1, 0)
                            avpair(jj - 1, 1)
                            es_q.popleft()
                        pull(2)
                    for hh in range(2):
                        ov = ov_pool.tile([D + 1, 512], f16, name="ov", tag="ov")
                        nc.vector.tensor_copy(ov, vacc[hh][0:D + 1, :])
                        h = 2 * pair + hh
                        nc.sync.dma_start(
                            out=recin[h:h + 1, :], in_=ov[D:D + 1, :])
                        ovs.append(ov)
                        pull(1)
                # batched reciprocal of the 4 denominators
                rec32a = rc_pool.tile([4, 512], f32, name="rec32a", tag="rec32a")
                rec32 = rc_pool.tile([4, 512], f32, name="rec32", tag="rec32")
                rec16 = rc_pool.tile([4, 512], f16, name="rec16", tag="rec16")
                nc.vector.tensor_copy(rec32a, recin)
                with nc.allow_low_precision(reason="softmax denom recip"):
                    nc.vector.reciprocal_approx_fast(out=rec32, in_=rec32a)
                    nc.vector.tensor_copy(rec16, rec32)

                def normalize_gen(outT=outT, ovs=ovs, rec16=rec16):
                    # deferred: the bc matmuls wait on the reciprocal chain,
                    # so they must not head-of-line-block next qc's scores
                    for h in range(4):
                        pair, odd = h // 2, h % 2
                        bc = ps_mm.tile([64, 512], f32, name="ps_bc", tag="mm")
                        nc.tensor.matmul(
                            bc, sel_sb[:, h * 64:(h + 1) * 64], rec16,
                            start=True, stop=True)
                        if not odd:
                            nc.vector.tensor_mul(
                                outT[pair][0:64, :], ovs[h][0:64, :], bc)
                        else:
                            ntmp = nt_pool.tile([64, 512], f16, name="ntmp",
                                                tag="ntmp")
                            nc.vector.tensor_mul(ntmp, ovs[h][0:64, :], bc)
                            nc.gpsimd.dma_start(
                                out=outT[pair][64:128, :], in_=ntmp)
                        yield

                filler.append(normalize_gen())
                filler.append(gen_oproj_qc(b, qc))

        # ---- software-pipelined emission ----
        QORDER = [(0, 1), (1, 0), (1, 1), (0, 2), (1, 2), (0, 3), (1, 3)]
        p1_0 = gen_qkv_part1(0)
        for _ in p1_0:
            pass
        need0 = {}
        for pair, qc in QORDER:
            g = gen_q_group(0, pair, qc)
            need0[(pair, qc)] = g
            filler.append(g)
        p1_1 = gen_qkv_part1(1)
        filler.append(p1_1)
        need1 = {(0, 0): p1_1}
        for pair, qc in QORDER:
            g = gen_q_group(1, pair, qc)
            need1[(pair, qc)] = g
            filler.append(g)
        gen_attn(0, need0)
        gen_attn(1, need1)
        while filler:
            pull(64)
    nc.compile()
    return nc


def _get_nc():
    if "nc" not in _cache:
        _cache["nc"] = _build_nc()
    return _cache["nc"]


def make_in_maps(x, W_Q, W_K, W_V, W_O):
    x = np.asarray(x, np.float32)
    W_Q = np.asarray(W_Q, np.float32)
    W_K = np.asarray(W_K, np.float32)
    W_V = np.asarray(W_V, np.float32)
    W_O = np.asarray(W_O, np.float32)
    xT = np.ascontiguousarray(x.transpose(0, 2, 1)).astype(np.float16)
    sel = np.zeros((4, 4 * 64), np.float16)
    for h in range(4):
        sel[h, h * 64:(h + 1) * 64] = 1.0
    in_maps = []
    for h in range(N_CORES):
        in_maps.append({
            "xT": xT,
            "wq": np.ascontiguousarray(W_Q[QD * h:QD * (h + 1), :].T).astype(np.float16),
            "wkv": np.ascontiguousarray(
                np.concatenate([W_K[D * h:D * (h + 1), :],
                                W_V[D * h:D * (h + 1), :]], axis=0).T).astype(np.float16),
            "wo": np.ascontiguousarray(W_O[:, QD * h:QD * (h + 1)].T).astype(np.float16),
            "ident": np.eye(128, dtype=np.float16),
            "ones": np.ones((1, 128), np.float16),
            "sel": sel,
        })
    return in_maps


def run_spmd(x, W_Q, W_K, W_V, W_O, **spmd_kwargs):
    from concourse.bass_utils import run_bass_kernel_spmd

    nc = _get_nc()
    in_maps = make_in_maps(x, W_Q, W_K, W_V, W_O)
    res = run_bass_kernel_spmd(nc, in_maps, list(range(N_CORES)), **spmd_kwargs)
    total = np.zeros((B, E, L), np.float32)
    for r in res.results:
        total += r["out"].astype(np.float32)
    out = np.ascontiguousarray(total.transpose(0, 2, 1))
    return out, res


def kernel(x, W_Q, W_K, W_V, W_O):
    out, _ = run_spmd(x, W_Q, W_K, W_V, W_O)
    return out


# revision 40
# speedup vs baseline: 1.0549x; 1.0549x over previous
"""GQA attention (B=2, L=2048, E=2048, 32 q-heads / 8 kv-heads, D=64) on 8 trn2
NeuronCores.

Sharding: tensor-parallel over kv-heads. Core h owns kv-head h: the 4 q-heads
4h..4h+3 (W_Q rows 256h:256h+256), W_K/W_V rows 64h:64h+64, and W_O columns
256h:256h+256. Each core computes a full-shape partial output
(x @ Wq_h -> attention -> @ Wo_h^T); the host sums the 8 partials (the
"all-reduce") and transposes back.

Device kernel layout notes:
  - fp16 operands everywhere (PSUM accumulation stays fp32): halves DMA/SBUF
    vs fp32 and keeps matmuls at the full 1 column/cycle stream rate.
  - x is fed pre-transposed (B, E, L) so the QKV projections consume it with
    the contraction dim (E) on partitions. Q/K are produced transposed
    (dims on partitions, tokens free), so scores are computed transposed:
    S^T[k, q] per 128-ktoken tile. Softmax needs no max pass (scores ~ N(0,1))
    and no transposes; the denominator comes from a ones-column appended to V.
  - Score matmuls have K=64 contraction, so two heads are packed into the PE
    array with row tiling: the even head of a pair streams through rows 0:64
    (stationary = K^T at partitions 0:64), the odd head through rows 64:128
    (stationary = a DMA-duplicated K^T at partitions 64:128, moving = odd Q
    which the QKV projection already leaves at partitions 64:128). The two
    matmuls execute concurrently -> ~2x on the score phase.
  - The attention inner loop is software-pipelined: scores for k-tile kt+1
    issue before the attn@V matmuls of k-tile kt, with filler PE work from
    other phases in between, so the PE never stalls on the ACT-engine exp.
  - Normalization: denominator rows ([1,512] each) are gathered by tiny
    SBUF->SBUF DMAs into a [4,512] tile per (b, q-chunk), one batched DVE
    reciprocal, then broadcast across 64 partitions with small one-hot
    matmuls on PE and applied with one DVE multiply per head.
  - Emission is software-pipelined across phases via a filler deque: QKV for
    batch b+1 and the per-q-chunk output projection slices (as soon as their
    outT q-chunk is normalized) are interleaved into the ACT(exp)-bound
    attention loop so the PE never idles long enough for the HAM clock gate
    to re-throttle it to 1.2 GHz.
"""

from collections import deque

import numpy as np

B, L, E = 2, 2048, 2048
HKV, D, G = 8, 64, 4          # kv heads (=cores), head dim, q-heads per core
QD = G * D                    # 256 q dims per core
N_CORES = 8
EC = E // 128                 # 16 contraction chunks for projections
NQC = L // 512                # 4 q-token chunks of 512
KT = L // 128                 # 16 k-token tiles of 128

_cache = {}


def _build_nc():
    import concourse.bass as bass
    import concourse.mybir as mybir
    import concourse.tile as tile
    from concourse import bacc
    from contextlib import ExitStack

    f32 = mybir.dt.float32
    f16 = mybir.dt.float16
    VW = D + 1  # V columns + the ones column (softmax denominator trick)

    nc = bacc.Bacc("TRN2", target_bir_lowering=False, debug=False)
    xT_d = nc.declare_dram_parameter("xT", [B, E, L], f16, isOutput=False)
    wq_d = nc.declare_dram_parameter("wq", [E, QD], f16, isOutput=False)
    wkv_d = nc.declare_dram_parameter("wkv", [E, 2 * D], f16, isOutput=False)
    wo_d = nc.declare_dram_parameter("wo", [QD, E], f16, isOutput=False)
    ident_d = nc.declare_dram_parameter("ident", [128, 128], f16, isOutput=False)
    ones_d = nc.declare_dram_parameter("ones", [1, 128], f16, isOutput=False)
    sel_d = nc.declare_dram_parameter("sel", [4, 4 * 64], f16, isOutput=False)
    out_d = nc.declare_dram_parameter("out", [B, E, L], f16, isOutput=True)

    with ExitStack() as ctx:
        tc = ctx.enter_context(tile.TileContext(nc))
        singles = ctx.enter_context(tc.tile_pool(name="singles", bufs=1))
        xt_pool = ctx.enter_context(tc.tile_pool(name="xtp", bufs=34))
        qt_pool = ctx.enter_context(tc.tile_pool(name="qtp", bufs=2))
        kv_pool = ctx.enter_context(tc.tile_pool(name="kvp", bufs=2))
        kd_pool = ctx.enter_context(tc.tile_pool(name="kdp", bufs=2))
        vs_pool = ctx.enter_context(tc.tile_pool(name="vsp", bufs=2))
        es_pool = ctx.enter_context(tc.tile_pool(name="esp", bufs=4))
        ov_pool = ctx.enter_context(tc.tile_pool(name="ovp", bufs=10))
        rc_pool = ctx.enter_context(tc.tile_pool(name="rcp", bufs=2))
        ot_pool = ctx.enter_context(tc.tile_pool(name="otp", bufs=2))
        nt_pool = ctx.enter_context(tc.tile_pool(name="ntp", bufs=3))
        st_pool = ctx.enter_context(tc.tile_pool(name="stp", bufs=4))
        ps_sc = ctx.enter_context(tc.tile_pool(name="pssc", bufs=2, space="PSUM"))
        ps_va = ctx.enter_context(tc.tile_pool(name="psva", bufs=2, space="PSUM"))
        ps_mm = ctx.enter_context(tc.tile_pool(name="psmm", bufs=2, space="PSUM"))

        # ---- static weights / constants ----
        # DMA order matters: ident (warm-up) and wkv (first projections)
        # must land before the bulk weights; wq/wo are issued later, after
        # the first half of x, from inside gen_qkv_part1(0).
        ident = singles.tile([128, 128], f16)
        nc.sync.dma_start(out=ident, in_=ident_d[:, :])
        sel_sb = singles.tile([4, 4 * 64], f16)
        nc.sync.dma_start(out=sel_sb, in_=sel_d[:, :])
        wkv_sb = singles.tile([128, EC * 2 * D], f16)
        nc.sync.dma_start(
            out=wkv_sb.rearrange("p (e m) -> p e m", e=EC),
            in_=wkv_d.rearrange("(e p) m -> p e m", p=128),
        )
        wq_sb = singles.tile([128, EC * QD], f16)  # e-chunk e at cols [e*256,(e+1)*256)
        wo_sb = [singles.tile([128, E], f16, name=f"wo_sb{kc}")
                 for kc in range(2)]

        # warm-up burst: keep the PE busy during the initial x DMA so the
        # HAM clock gate ramps to 2.4 GHz before the real work arrives
        for _ in range(48):
            wps = ps_mm.tile([128, 128], f16, name="warm", tag="mm")
            nc.tensor.transpose(wps, ident, ident)

        state = {}

        def qkv_group(b, xts, m, hf, t):
            """One [128,512] projection group: accumulate over 16 e-chunks."""
            n = hf * 2 + t
            nsl = slice(n * 512, (n + 1) * 512)
            ps = ps_mm.tile([128, 512], f32, name="ps_qkv", tag="mm")
            for e in range(EC):
                if m < 2:
                    lhsT = wq_sb[:, e * QD + m * 128: e * QD + (m + 1) * 128]
                else:
                    lhsT = wkv_sb[:, e * 2 * D:(e + 1) * 2 * D]
                nc.tensor.matmul(
                    ps, lhsT, xts[hf][e][:, t * 512:(t + 1) * 512],
                    start=(e == 0), stop=(e == EC - 1),
                )
                yield
            st = state[b]
            dst = st["qpair"][m] if m < 2 else st["kvT"]
            nc.vector.tensor_copy(dst[:, nsl], ps)
            if m == 2:
                # duplicate K^T to partitions 64:128 for the row-tiled
                # odd-head score matmuls
                nc.sync.dma_start(
                    out=st["kdup"][64:128, nsl], in_=st["kvT"][0:64, nsl])

        def gen_qkv_part1(b):
            """x DMAs, K/V projection (+K dup), V transpose, first q group."""
            qpair = [qt_pool.tile([128, L], f16, name=f"qpair{p}", tag=f"qpair{p}")
                     for p in range(2)]
            kvT = kv_pool.tile([128, L], f16, name="kvT", tag="kvT")
            kdup = kd_pool.tile([128, L], f16, name="kdup", tag="kdup")
            v_sb = vs_pool.tile([128, KT * VW], f16, name="v_sb", tag="v_sb")
            xts = {}
            state[b] = dict(qpair=qpair, kvT=kvT, kdup=kdup, v_sb=v_sb, xts=xts)
            # ones column (denominator trick) via partition-broadcast DMA
            ones_bcast = bass.AP(
                tensor=ones_d[0:1, 0:KT].tensor, offset=0,
                ap=[[0, 128], [1, KT]])
            nc.sync.dma_start(
                out=v_sb.rearrange("p (k c) -> p k c", c=VW)[:, :, D],
                in_=ones_bcast)
            for hf in range(2):
                xts[hf] = []
                for e in range(EC):
                    xt = xt_pool.tile([128, 1024], f16, name=f"xt_{e}", tag="xt")
                    nc.sync.dma_start(
                        out=xt,
                        in_=xT_d[b, e * 128:(e + 1) * 128,
                                 hf * 1024:(hf + 1) * 1024],
                    )
                    xts[hf].append(xt)
                if b == 0 and hf == 1:
                    # bulk weights ride behind x; wq arrives just before the
                    # first q projection group needs it
                    nc.sync.dma_start(
                        out=wq_sb.rearrange("p (e m) -> p e m", e=EC),
                        in_=wq_d.rearrange("(e p) m -> p e m", p=128),
                    )
                    for kc in range(2):
                        nc.sync.dma_start(
                            out=wo_sb[kc],
                            in_=wo_d[kc * 128:(kc + 1) * 128, :])
            for hf in range(2):
                for t in range(2):
                    yield from qkv_group(b, xts, 2, hf, t)
            for kt in range(KT):
                psv = ps_mm.tile([128, 64], f16, name="ps_vt", tag="mm")
                nc.tensor.transpose(
                    psv, kvT[64:128, kt * 128:(kt + 1) * 128],
                    ident[64:128, 64:128])
                nc.vector.tensor_copy(
                    v_sb[:, kt * VW: kt * VW + D], psv)
                yield
            yield from qkv_group(b, xts, 0, 0, 0)  # (pair0, qc0)

        def gen_q_group(b, pair, qc):
            """One q projection group; attention (qc,pair) guards on it."""
            yield from qkv_group(b, state[b]["xts"], pair, qc // 2, qc % 2)

        filler = deque()

        def pull(k):
            n = 0
            while n < k and filler:
                try:
                    next(filler[0])
                    n += 1
                except StopIteration:
                    filler.popleft()

        def drain_through(gen):
            """Emit deque items in order until `gen` is exhausted."""
            if gen not in filler:
                return
            while filler:
                head = filler[0]
                try:
                    next(head)
                except StopIteration:
                    filler.popleft()
                    if head is gen:
                        return

        def gen_oproj_qc(b, qc):
            """Output projection for one 512-token chunk of batch b."""
            outT = state.pop((b, "outT", qc))
            nsl = slice(qc * 512, (qc + 1) * 512)
            for m in range(EC):
                msl = slice(m * 128, (m + 1) * 128)
                ps = ps_mm.tile([128, 512], f32, name="ps_op", tag="mm")
                for kc in range(2):
                    nc.tensor.matmul(
                        ps, wo_sb[kc][:, msl], outT[kc][:, :],
                        start=(kc == 0), stop=(kc == 1),
                    )
                    yield
                stg = st_pool.tile([128, 512], f16, name="stg", tag="stg")
                nc.vector.tensor_copy(stg, ps)
                nc.gpsimd.dma_start(out=out_d[b, msl, nsl], in_=stg)

        def gen_attn(b, need):
            """Attention for batch b; need[(pair,qc)] = q-group generators
            that must be fully emitted before that section's scores."""
            st = state[b]
            qpair, kvT, kdup, v_sb = (
                st["qpair"], st["kvT"], st["kdup"], st["v_sb"])
            for qc in range(NQC):
                qsl = slice(qc * 512, (qc + 1) * 512)
                outT = [ot_pool.tile([128, 512], f16, name=f"oT{p}",
                                     tag=f"outT{p}q{qc}")
                        for p in range(2)]
                state[(b, "outT", qc)] = outT
                recin = rc_pool.tile([4, 512], f16, name="recin", tag="recin")
                ovs = []
                for pair in range(2):
                    g = need.pop((pair, qc), None)
                    if g is not None:
                        drain_through(g)
                    vacc = [
                        ps_va.tile([VW, 512], f32, name=f"vacc{hh}",
                                   tag="vacc")
                        for hh in range(2)]

                    def spair(jj, r):
                        # per head: scores for k-tile 2jj+r; the even head
                        # streams PE rows 0:64, the odd head rows 64:128 --
                        # adjacent emission makes the pair run concurrently
                        kt = 2 * jj + r
                        ksl = slice(kt * 128, (kt + 1) * 128)
                        csl = slice(r * 512, (r + 1) * 512)
                        nc.tensor.matmul(
                            ssc[0][:, csl], kvT[0:64, ksl],
                            qpair[pair][0:64, qsl], start=True, stop=True)
                        nc.tensor.matmul(
                            ssc[1][:, csl], kdup[64:128, ksl],
                            qpair[pair][64:128, qsl], start=True, stop=True)

                    def avpair(jj, r):
                        es2 = es_q[0]
                        kt = 2 * jj + r
                        for hh in range(2):
                            nc.tensor.matmul(
                                vacc[hh][0:VW, :],
                                v_sb[:, kt * VW:(kt + 1) * VW],
                                es2[hh][:, r * 512:(r + 1) * 512],
                                start=(kt == 0), stop=(kt == KT - 1),
                            )

                    es_q = deque()
                    for jj in range(KT // 2 + 1):
                        if jj < KT // 2:
                            ssc = [ps_sc.tile([128, 1024], f32,
                                              name=f"ssc{hh}", tag="ssc")
                                   for hh in range(2)]
                            spair(jj, 0)
                            spair(jj, 1)
                            es2 = []
                            for hh in range(2):
                                es = es_pool.tile([128, 1024], f16, name="es",
                                                  tag="es")
                                nc.scalar.activation(
                                    es, ssc[hh],
                                    mybir.ActivationFunctionType.Exp,
                                    scale=0.125)
                                es2.append(es)
                            es_q.append(es2)
                        pull(2)
                        if jj > 0:
                            avpair(jj - 1, 0)
                            avpair(jj - 1, 1)
                            es_q.popleft()
                        pull(2)
                    for hh in range(2):
                        ov = ov_pool.tile([D + 1, 512], f16, name="ov", tag="ov")
                        nc.vector.tensor_copy(ov, vacc[hh][0:D + 1, :])
                        h = 2 * pair + hh
                        nc.sync.dma_start(
                            out=recin[h:h + 1, :], in_=ov[D:D + 1, :])
                        ovs.append(ov)
                        pull(1)
                # batched reciprocal of the 4 denominators
                rec32a = rc_pool.tile([4, 512], f32, name="rec32a", tag="rec32a")
                rec32 = rc_pool.tile([4, 512], f32, name="rec32", tag="rec32")
                rec16 = rc_pool.tile([4, 512], f16, name="rec16", tag="rec16")
                nc.vector.tensor_copy(rec32a, recin)
                with nc.allow_low_precision(reason="softmax denom recip"):
                    nc.vector.reciprocal_approx_fast(out=rec32, in_=rec32a)
                    nc.vector.tensor_copy(rec16, rec32)

                def normalize_gen(outT=outT, ovs=ovs, rec16=rec16):
                    # deferred: the bc matmuls wait on the reciprocal chain,
                    # so they must not head-of-line-block next qc's scores
                    for h in range(4):
                        pair, odd = h // 2, h % 2
                        bc = ps_mm.tile([64, 512], f32, name="ps_bc", tag="mm")
                        nc.tensor.matmul(
                            bc, sel_sb[:, h * 64:(h + 1) * 64], rec16,
                            start=True, stop=True)
                        if not odd:
                            nc.vector.tensor_mul(
                                outT[pair][0:64, :], ovs[h][0:64, :], bc)
                        else:
                            ntmp = nt_pool.tile([64, 512], f16, name="ntmp",
                                                tag="ntmp")
                            nc.vector.tensor_mul(ntmp, ovs[h][0:64, :], bc)
                            nc.gpsimd.dma_start(
                                out=outT[pair][64:128, :], in_=ntmp)
                        yield

                filler.appendleft(normalize_gen())
                filler.append(gen_oproj_qc(b, qc))

        # ---- software-pipelined emission ----
        QORDER = [(0, 1), (1, 0), (1, 1), (0, 2), (1, 2), (0, 3), (1, 3)]
        p1_0 = gen_qkv_part1(0)
        for _ in p1_0:
            pass
        need0 = {}
        for pair, qc in QORDER:
            g = gen_q_group(0, pair, qc)
            need0[(pair, qc)] = g
            filler.append(g)
        p1_1 = gen_qkv_part1(1)
        filler.append(p1_1)
        need1 = {(0, 0): p1_1}
        for pair, qc in QORDER:
            g = gen_q_group(1, pair, qc)
            need1[(pair, qc)] = g
            filler.append(g)
        gen_attn(0, need0)
        gen_attn(1, need1)
        while filler:
            pull(64)
    nc.compile()
    return nc


def _get_nc():
    if "nc" not in _cache:
        _cache["nc"] = _build_nc()
    return _cache["nc"]


def make_in_maps(x, W_Q, W_K, W_V, W_O):
    x = np.asarray(x, np.float32)
    W_Q = np.asarray(W_Q, np.float32)
    W_K = np.asarray(W_K, np.float32)
    W_V = np.asarray(W_V, np.float32)
    W_O = np.asarray(W_O, np.float32)
    xT = np.ascontiguousarray(x.transpose(0, 2, 1)).astype(np.float16)
    sel = np.zeros((4, 4 * 64), np.float16)
    for h in range(4):
        sel[h, h * 64:(h + 1) * 64] = 1.0
    in_maps = []
    for h in range(N_CORES):
        in_maps.append({
            "xT": xT,
            "wq": np.ascontiguousarray(W_Q[QD * h:QD * (h + 1), :].T).astype(np.float16),
            "wkv": np.ascontiguousarray(
                np.concatenate([W_K[D * h:D * (h + 1), :],
                                W_V[D * h:D * (h + 1), :]], axis=0).T).astype(np.float16),
            "wo": np.ascontiguousarray(W_O[:, QD * h:QD * (h + 1)].T).astype(np.float16),
            "ident": np.eye(128, dtype=np.float16),
            "ones": np.ones((1, 128), np.float16),
            "sel": sel,
        })
    return in_maps


def run_spmd(x, W_Q, W_K, W_V, W_O, **spmd_kwargs):
    from concourse.bass_utils import run_bass_kernel_spmd

    nc = _get_nc()
    in_maps = make_in_maps(x, W_Q, W_K, W_V, W_O)
    res = run_bass_kernel_spmd(nc, in_maps, list(range(N_CORES)), **spmd_kwargs)
    total = np.zeros((B, E, L), np.float32)
    for r in res.results:
        total += r["out"].astype(np.float32)
    out = np.ascontiguousarray(total.transpose(0, 2, 1))
    return out, res


def kernel(x, W_Q, W_K, W_V, W_O):
    out, _ = run_spmd(x, W_Q, W_K, W_V, W_O)
    return out


# revision 49
# speedup vs baseline: 1.0594x; 1.0042x over previous
"""GQA attention (B=2, L=2048, E=2048, 32 q-heads / 8 kv-heads, D=64) on 8 trn2
NeuronCores.

Sharding: tensor-parallel over kv-heads. Core h owns kv-head h: the 4 q-heads
4h..4h+3 (W_Q rows 256h:256h+256), W_K/W_V rows 64h:64h+64, and W_O columns
256h:256h+256. Each core computes a full-shape partial output
(x @ Wq_h -> attention -> @ Wo_h^T); the host sums the 8 partials (the
"all-reduce") and transposes back.

Device kernel layout notes:
  - fp16 operands everywhere (PSUM accumulation stays fp32): halves DMA/SBUF
    vs fp32 and keeps matmuls at the full 1 column/cycle stream rate.
  - x is fed pre-transposed (B, E, L) so the QKV projections consume it with
    the contraction dim (E) on partitions. Q/K are produced transposed
    (dims on partitions, tokens free), so scores are computed transposed:
    S^T[k, q] per 128-ktoken tile. Softmax needs no max pass (scores ~ N(0,1))
    and no transposes; the denominator comes from a ones-column appended to V.
  - Score matmuls have K=64 contraction, so two heads are packed into the PE
    array with row tiling: the even head of a pair streams through rows 0:64
    (stationary = K^T at partitions 0:64), the odd head through rows 64:128
    (stationary = a DMA-duplicated K^T at partitions 64:128, moving = odd Q
    which the QKV projection already leaves at partitions 64:128). The two
    matmuls execute concurrently -> ~2x on the score phase.
  - The attention inner loop is software-pipelined: scores for k-tile kt+1
    issue before the attn@V matmuls of k-tile kt, with filler PE work from
    other phases in between, so the PE never stalls on the ACT-engine exp.
  - Normalization: denominator rows ([1,512] each) are gathered by tiny
    SBUF->SBUF DMAs into a [4,512] tile per (b, q-chunk), one batched DVE
    reciprocal, then broadcast across 64 partitions with small one-hot
    matmuls on PE and applied with one DVE multiply per head.
  - Emission is software-pipelined across phases via a filler deque: QKV for
    batch b+1 and the per-q-chunk output projection slices (as soon as their
    outT q-chunk is normalized) are interleaved into the ACT(exp)-bound
    attention loop so the PE never idles long enough for the HAM clock gate
    to re-throttle it to 1.2 GHz.
"""

from collections import deque

import numpy as np

B, L, E = 2, 2048, 2048
HKV, D, G = 8, 64, 4          # kv heads (=cores), head dim, q-heads per core
QD = G * D                    # 256 q dims per core
N_CORES = 8
EC = E // 128                 # 16 contraction chunks for projections
NQC = L // 512                # 4 q-token chunks of 512
KT = L // 128                 # 16 k-token tiles of 128

_cache = {}


def _build_nc():
    import concourse.bass as bass
    import concourse.mybir as mybir
    import concourse.tile as tile
    from concourse import bacc
    from contextlib import ExitStack

    f32 = mybir.dt.float32
    f16 = mybir.dt.float16
    VW = D + 1  # V columns + the ones column (softmax denominator trick)

    nc = bacc.Bacc("TRN2", target_bir_lowering=False, debug=False)
    xT_d = nc.declare_dram_parameter("xT", [B, E, L], f16, isOutput=False)
    wq_d = nc.declare_dram_parameter("wq", [E, QD], f16, isOutput=False)
    wkv_d = nc.declare_dram_parameter("wkv", [E, 2 * D], f16, isOutput=False)
    wo_d = nc.declare_dram_parameter("wo", [QD, E], f16, isOutput=False)
    ident_d = nc.declare_dram_parameter("ident", [128, 128], f16, isOutput=False)
    ones_d = nc.declare_dram_parameter("ones", [1, 128], f16, isOutput=False)
    sel_d = nc.declare_dram_parameter("sel", [2, 2 * 64], f16, isOutput=False)
    out_d = nc.declare_dram_parameter("out", [B, E, L], f16, isOutput=True)

    with ExitStack() as ctx:
        tc = ctx.enter_context(tile.TileContext(nc))
        singles = ctx.enter_context(tc.tile_pool(name="singles", bufs=1))
        xt_pool = ctx.enter_context(tc.tile_pool(name="xtp", bufs=34))
        qt_pool = ctx.enter_context(tc.tile_pool(name="qtp", bufs=2))
        kv_pool = ctx.enter_context(tc.tile_pool(name="kvp", bufs=2))
        kd_pool = ctx.enter_context(tc.tile_pool(name="kdp", bufs=2))
        vs_pool = ctx.enter_context(tc.tile_pool(name="vsp", bufs=2))
        es_pool = ctx.enter_context(tc.tile_pool(name="esp", bufs=4))
        ov_pool = ctx.enter_context(tc.tile_pool(name="ovp", bufs=10))
        rc_pool = ctx.enter_context(tc.tile_pool(name="rcp", bufs=3))
        ot_pool = ctx.enter_context(tc.tile_pool(name="otp", bufs=2))
        nt_pool = ctx.enter_context(tc.tile_pool(name="ntp", bufs=3))
        st_pool = ctx.enter_context(tc.tile_pool(name="stp", bufs=4))
        ps_sc = ctx.enter_context(tc.tile_pool(name="pssc", bufs=2, space="PSUM"))
        ps_va = ctx.enter_context(tc.tile_pool(name="psva", bufs=2, space="PSUM"))
        ps_mm = ctx.enter_context(tc.tile_pool(name="psmm", bufs=2, space="PSUM"))

        # ---- static weights / constants ----
        # DMA order matters: ident (warm-up) and wkv (first projections)
        # must land before the bulk weights; wq/wo are issued later, after
        # the first half of x, from inside gen_qkv_part1(0).
        ident = singles.tile([128, 128], f16)
        nc.sync.dma_start(out=ident, in_=ident_d[:, :])
        sel_sb = singles.tile([2, 2 * 64], f16)
        nc.sync.dma_start(out=sel_sb, in_=sel_d[:, :])
        wkv_sb = singles.tile([128, EC * 2 * D], f16)
        nc.sync.dma_start(
            out=wkv_sb.rearrange("p (e m) -> p e m", e=EC),
            in_=wkv_d.rearrange("(e p) m -> p e m", p=128),
        )
        wq_sb = singles.tile([128, EC * QD], f16)  # e-chunk e at cols [e*256,(e+1)*256)
        wo_sb = [singles.tile([128, E], f16, name=f"wo_sb{kc}")
                 for kc in range(2)]

        # warm-up burst: keep the PE busy during the initial x DMA so the
        # HAM clock gate ramps to 2.4 GHz before the real work arrives
        for _ in range(48):
            wps = ps_mm.tile([128, 128], f16, name="warm", tag="mm")
            nc.tensor.transpose(wps, ident, ident)

        state = {}

        def qkv_group(b, xts, m, hf, t):
            """One [128,512] projection group: accumulate over 16 e-chunks."""
            n = hf * 2 + t
            nsl = slice(n * 512, (n + 1) * 512)
            ps = ps_mm.tile([128, 512], f32, name="ps_qkv", tag="mm")
            for e in range(EC):
                if m < 2:
                    lhsT = wq_sb[:, e * QD + m * 128: e * QD + (m + 1) * 128]
                else:
                    lhsT = wkv_sb[:, e * 2 * D:(e + 1) * 2 * D]
                nc.tensor.matmul(
                    ps, lhsT, xts[hf][e][:, t * 512:(t + 1) * 512],
                    start=(e == 0), stop=(e == EC - 1),
                )
                yield
            st = state[b]
            dst = st["qpair"][m] if m < 2 else st["kvT"]
            nc.vector.tensor_copy(dst[:, nsl], ps)
            if m == 2:
                # duplicate K^T to partitions 64:128 for the row-tiled
                # odd-head score matmuls
                nc.sync.dma_start(
                    out=st["kdup"][64:128, nsl], in_=st["kvT"][0:64, nsl])

        def gen_qkv_part1(b):
            """x DMAs, K/V projection (+K dup), V transpose, first q group."""
            qpair = [qt_pool.tile([128, L], f16, name=f"qpair{p}", tag=f"qpair{p}")
                     for p in range(2)]
            kvT = kv_pool.tile([128, L], f16, name="kvT", tag="kvT")
            kdup = kd_pool.tile([128, L], f16, name="kdup", tag="kdup")
            v_sb = vs_pool.tile([128, KT * VW], f16, name="v_sb", tag="v_sb")
            xts = {}
            state[b] = dict(qpair=qpair, kvT=kvT, kdup=kdup, v_sb=v_sb, xts=xts)
            # ones column (denominator trick) via partition-broadcast DMA
            ones_bcast = bass.AP(
                tensor=ones_d[0:1, 0:KT].tensor, offset=0,
                ap=[[0, 128], [1, KT]])
            nc.sync.dma_start(
                out=v_sb.rearrange("p (k c) -> p k c", c=VW)[:, :, D],
                in_=ones_bcast)
            for hf in range(2):
                xts[hf] = []
                for e in range(EC):
                    xt = xt_pool.tile([128, 1024], f16, name=f"xt_{e}", tag="xt")
                    nc.sync.dma_start(
                        out=xt,
                        in_=xT_d[b, e * 128:(e + 1) * 128,
                                 hf * 1024:(hf + 1) * 1024],
                    )
                    xts[hf].append(xt)
                if b == 0 and hf == 1:
                    # bulk weights ride behind x; wq arrives just before the
                    # first q projection group needs it
                    nc.sync.dma_start(
                        out=wq_sb.rearrange("p (e m) -> p e m", e=EC),
                        in_=wq_d.rearrange("(e p) m -> p e m", p=128),
                    )
                    for kc in range(2):
                        nc.sync.dma_start(
                            out=wo_sb[kc],
                            in_=wo_d[kc * 128:(kc + 1) * 128, :])
            for hf in range(2):
                for t in range(2):
                    yield from qkv_group(b, xts, 2, hf, t)
            for kt in range(KT):
                psv = ps_mm.tile([128, 64], f16, name="ps_vt", tag="mm")
                nc.tensor.transpose(
                    psv, kvT[64:128, kt * 128:(kt + 1) * 128],
                    ident[64:128, 64:128])
                nc.vector.tensor_copy(
                    v_sb[:, kt * VW: kt * VW + D], psv)
                yield
            yield from qkv_group(b, xts, 0, 0, 0)  # (pair0, qc0)

        def gen_q_group(b, pair, qc):
            """One q projection group; attention (qc,pair) guards on it."""
            yield from qkv_group(b, state[b]["xts"], pair, qc // 2, qc % 2)

        filler = deque()

        def pull(k):
            n = 0
            while n < k and filler:
                try:
                    next(filler[0])
                    n += 1
                except StopIteration:
                    filler.popleft()

        def drain_through(gen):
            """Emit deque items in order until `gen` is exhausted."""
            if gen not in filler:
                return
            while filler:
                head = filler[0]
                try:
                    next(head)
                except StopIteration:
                    filler.popleft()
                    if head is gen:
                        return

        def gen_oproj_qc(b, qc):
            """Output projection for one 512-token chunk of batch b."""
            outT = state.pop((b, "outT", qc))
            nsl = slice(qc * 512, (qc + 1) * 512)
            for m in range(EC):
                msl = slice(m * 128, (m + 1) * 128)
                ps = ps_mm.tile([128, 512], f32, name="ps_op", tag="mm")
                for kc in range(2):
                    nc.tensor.matmul(
                        ps, wo_sb[kc][:, msl], outT[kc][:, :],
                        start=(kc == 0), stop=(kc == 1),
                    )
                    yield
                stg = st_pool.tile([128, 512], f16, name="stg", tag="stg")
                nc.vector.tensor_copy(stg, ps)
                nc.gpsimd.dma_start(out=out_d[b, msl, nsl], in_=stg)

        def gen_attn(b, need):
            """Attention for batch b; need[(pair,qc)] = q-group generators
            that must be fully emitted before that section's scores."""
            st = state[b]
            qpair, kvT, kdup, v_sb = (
                st["qpair"], st["kvT"], st["kdup"], st["v_sb"])
            for qc in range(NQC):
                qsl = slice(qc * 512, (qc + 1) * 512)
                outT = [ot_pool.tile([128, 512], f16, name=f"oT{p}",
                                     tag=f"outT{p}q{qc}")
                        for p in range(2)]
                state[(b, "outT", qc)] = outT
                recs = {}
                ovs = []
                for pair in range(2):
                    g = need.pop((pair, qc), None)
                    if g is not None:
                        drain_through(g)
                    vacc = [
                        ps_va.tile([VW, 512], f32, name=f"vacc{hh}",
                                   tag="vacc")
                        for hh in range(2)]

                    def spair(jj, r):
                        # per head: scores for k-tile 2jj+r; the even head
                        # streams PE rows 0:64, the odd head rows 64:128 --
                        # adjacent emission makes the pair run concurrently
                        kt = 2 * jj + r
                        ksl = slice(kt * 128, (kt + 1) * 128)
                        csl = slice(r * 512, (r + 1) * 512)
                        nc.tensor.matmul(
                            ssc[0][:, csl], kvT[0:64, ksl],
                            qpair[pair][0:64, qsl], start=True, stop=True)
                        nc.tensor.matmul(
                            ssc[1][:, csl], kdup[64:128, ksl],
                            qpair[pair][64:128, qsl], start=True, stop=True)

                    def avpair(jj, r):
                        es2 = es_q[0]
                        kt = 2 * jj + r
                        for hh in range(2):
                            nc.tensor.matmul(
                                vacc[hh][0:VW, :],
                                v_sb[:, kt * VW:(kt + 1) * VW],
                                es2[hh][:, r * 512:(r + 1) * 512],
                                start=(kt == 0), stop=(kt == KT - 1),
                            )

                    es_q = deque()
                    for jj in range(KT // 2 + 1):
                        if jj < KT // 2:
                            ssc = [ps_sc.tile([128, 1024], f32,
                                              name=f"ssc{hh}", tag="ssc")
                                   for hh in range(2)]
                            spair(jj, 0)
                            spair(jj, 1)
                            es2 = []
                            for hh in range(2):
                                es = es_pool.tile([128, 1024], f16, name="es",
                                                  tag="es")
                                nc.scalar.activation(
                                    es, ssc[hh],
                                    mybir.ActivationFunctionType.Exp,
                                    scale=0.125)
                                es2.append(es)
                            es_q.append(es2)
                        pull(2)
                        if jj > 0:
                            avpair(jj - 1, 0)
                            avpair(jj - 1, 1)
                            es_q.popleft()
                        pull(2)
                    recin = rc_pool.tile([2, 512], f16, name="recin",
                                         tag="recin")
                    for hh in range(2):
                        ov = ov_pool.tile([D + 1, 512], f16, name="ov", tag="ov")
                        nc.vector.tensor_copy(ov, vacc[hh][0:D + 1, :])
                        nc.sync.dma_start(
                            out=recin[hh:hh + 1, :], in_=ov[D:D + 1, :])
                        ovs.append(ov)
                        pull(1)
                    # reciprocal of this pair's two denominators (split per
                    # pair so the qc-boundary chain is short)
                    rec32a = rc_pool.tile([2, 512], f32, name="rec32a",
                                          tag="rec32a")
                    rec32 = rc_pool.tile([2, 512], f32, name="rec32",
                                         tag="rec32")
                    rec16 = rc_pool.tile([2, 512], f16, name="rec16",
                                         tag="rec16")
                    nc.vector.tensor_copy(rec32a, recin)
                    with nc.allow_low_precision(reason="softmax denom recip"):
                        nc.vector.reciprocal_approx_fast(
                            out=rec32, in_=rec32a)
                        nc.vector.tensor_copy(rec16, rec32)
                    recs[pair] = rec16

                def normalize_gen(outT=outT, ovs=ovs, recs=recs):
                    # deferred: the bc matmuls wait on the reciprocal chain,
                    # so they must not head-of-line-block next qc's scores
                    for h in range(4):
                        pair, odd = h // 2, h % 2
                        bc = ps_mm.tile([64, 512], f32, name="ps_bc", tag="mm")
                        nc.tensor.matmul(
                            bc, sel_sb[:, odd * 64:(odd + 1) * 64],
                            recs[pair], start=True, stop=True)
                        if not odd:
                            nc.vector.tensor_mul(
                                outT[pair][0:64, :], ovs[h][0:64, :], bc)
                        else:
                            ntmp = nt_pool.tile([64, 512], f16, name="ntmp",
                                                tag="ntmp")
                            nc.vector.tensor_mul(ntmp, ovs[h][0:64, :], bc)
                            nc.gpsimd.dma_start(
                                out=outT[pair][64:128, :], in_=ntmp)
                        yield

                filler.appendleft(normalize_gen())
                if b == 1 and qc == 2:
                    # held back: interleaved with qc3's o-proj in the tail
                    # drain so the PSUM-ring copy latency hides
                    state["held_oproj"] = gen_oproj_qc(b, qc)
                else:
                    filler.append(gen_oproj_qc(b, qc))

        # ---- software-pipelined emission ----
        QORDER = [(0, 1), (1, 0), (1, 1), (0, 2), (1, 2), (0, 3), (1, 3)]
        p1_0 = gen_qkv_part1(0)
        for _ in p1_0:
            pass
        need0 = {}
        for pair, qc in QORDER:
            g = gen_q_group(0, pair, qc)
            need0[(pair, qc)] = g
            filler.append(g)
        p1_1 = gen_qkv_part1(1)
        filler.append(p1_1)
        need1 = {(0, 0): p1_1}
        for pair, qc in QORDER:
            g = gen_q_group(1, pair, qc)
            need1[(pair, qc)] = g
            filler.append(g)
        gen_attn(0, need0)
        gen_attn(1, need1)
        held = state.pop("held_oproj", None)
        while filler or held is not None:
            if held is not None:
                try:
                    next(held)
                except StopIteration:
                    held = None
            pull(1)
    nc.compile()
    return nc


def _get_nc():
    if "nc" not in _cache:
        _cache["nc"] = _build_nc()
    return _cache["nc"]


def make_in_maps(x, W_Q, W_K, W_V, W_O):
    x = np.asarray(x, np.float32)
    W_Q = np.asarray(W_Q, np.float32)
    W_K = np.asarray(W_K, np.float32)
    W_V = np.asarray(W_V, np.float32)
    W_O = np.asarray(W_O, np.float32)
    xT = np.ascontiguousarray(x.transpose(0, 2, 1)).astype(np.float16)
    sel = np.zeros((2, 2 * 64), np.float16)
    for h in range(2):
        sel[h, h * 64:(h + 1) * 64] = 1.0
    in_maps = []
    for h in range(N_CORES):
        in_maps.append({
            "xT": xT,
            "wq": np.ascontiguousarray(W_Q[QD * h:QD * (h + 1), :].T).astype(np.float16),
            "wkv": np.ascontiguousarray(
                np.concatenate([W_K[D * h:D * (h + 1), :],
                                W_V[D * h:D * (h + 1), :]], axis=0).T).astype(np.float16),
            "wo": np.ascontiguousarray(W_O[:, QD * h:QD * (h + 1)].T).astype(np.float16),
            "ident": np.eye(128, dtype=np.float16),
            "ones": np.ones((1, 128), np.float16),
            "sel": sel,
        })
    return in_maps


def run_spmd(x, W_Q, W_K, W_V, W_O, **spmd_kwargs):
    from concourse.bass_utils import run_bass_kernel_spmd

    nc = _get_nc()
    in_maps = make_in_maps(x, W_Q, W_K, W_V, W_O)
    res = run_bass_kernel_spmd(nc, in_maps, list(range(N_CORES)), **spmd_kwargs)
    total = np.zeros((B, E, L), np.float32)
    for r in res.results:
        total += r["out"].astype(np.float32)
    out = np.ascontiguousarray(total.transpose(0, 2, 1))
    return out, res


def kernel(x, W_Q, W_K, W_V, W_O):
    out, _ = run_spmd(x, W_Q, W_K, W_V, W_O)
    return out
